# revision 20
# baseline (speedup 1.0000x reference)
"""MultiHeadCrossAttention kernel for 8 trn2 NeuronCores.

Reference computation (fp32, per batch b):
    q = Q[b] @ W_q.T ; k = K[b] @ W_k.T ; v = V[b] @ W_v.T      (heads on columns)
    per head h: S = (q_h @ k_h.T) / 8 ; E = exp(S); A = E / E.sum(-1)
    out[b] = concat_h(A @ v_h) @ W_o.T ; rows with mask==0 zeroed

Sharding: 8 cores = (batch b in {0,1}) x (head-group hg in {0..3}, 4 heads each).
Each core computes a partial output  out_part[b] = concat(heads hg) @ W_o[:, cols].T
and the host sums the 4 partials per batch (bf16 partials, fp32 host sum).

Design: the kernel is ScalarE-bound (exp over 4 heads x 2048 x 2048 = 16.8M
elements at 1 elem/lane/cycle @ 1.2 GHz ~= 147 us).  Everything else is
scheduled to hide under the exp stream:
  - The attention j-loop emits scores (row-tiled concurrent matmul pair, two
    heads at PE rows 0:64/64:128), the exp ACTIVATE, then background work,
    then the PV matmuls (augmented-V: stationary col 0 = ones accumulates the
    softmax denominator in PSUM row 0).
  - Background work (remaining projections, W_o, DMA posts) is emitted as
    generator "units" that yield every ~2 matmuls, pumped at most 2 yields per
    j-step so the PE FIFO never delays the next scores pair by more than
    ~0.9us.  Deadline fields force-drain units whose results the next step
    needs.
  - Deep e-tile buffering lets ScalarE run ahead of PV when the PE transiently
    falls behind (e.g. late V-chunk DMAs).
  - DMA posts cost ~0.65us each on the posting engine's queue and transfers
    are slow (~130-250 GB/s effective), so the first K/Q chunks go on the
    scalar ring (idle before the exp stream starts) while everything else
    staggers through the sync ring in need-order.
  - Reciprocal broadcast across partitions via a K=1 PE outer-product.
"""

import numpy as np
import ml_dtypes

import concourse.bass as bass
import concourse.bacc as bacc
import concourse.mybir as mybir
import concourse.tile as tile
from contextlib import ExitStack

F32 = mybir.dt.float32
BF16 = mybir.dt.bfloat16
AF = mybir.ActivationFunctionType

B = 2
SEQ = 2048          # Sq == Sk
D = 1024            # model dim
DL = 256            # local head dims per core (4 heads x 64)
HL = 4              # local heads
DH = 64             # head dim
NCORES = 8

_PROGRAM = None


def build_program():
    nc = bacc.Bacc("TRN2", target_bir_lowering=False)

    # x inputs host-packed as [p, c*8+a, s]: element (model a*128+p, seq c*512+s)
    xq = nc.declare_dram_parameter("xq", [128, 32, 512], BF16, isOutput=False)
    xk = nc.declare_dram_parameter("xk", [128, 32, 512], BF16, isOutput=False)
    xv = nc.declare_dram_parameter("xv", [128, 32, 512], BF16, isOutput=False)
    # weights host-packed [p, a, d]: element (model a*128+p, d)
    wq = nc.declare_dram_parameter("wq", [128, 8, DL], BF16, isOutput=False)
    wk = nc.declare_dram_parameter("wk", [128, 8, DL], BF16, isOutput=False)
    wv = nc.declare_dram_parameter("wv", [128, 8, DL], BF16, isOutput=False)
    wo = nc.declare_dram_parameter("wo", [128, 2, D], BF16, isOutput=False)
    maskf = nc.declare_dram_parameter("maskf", [128, SEQ // 128], F32, isOutput=False)
    out_part = nc.declare_dram_parameter("out_part", [SEQ, D], BF16, isOutput=True)

    with tile.TileContext(nc) as tc, ExitStack() as ctx:
        const = ctx.enter_context(tc.tile_pool(name="const", bufs=1))
        proj = ctx.enter_context(tc.tile_pool(name="proj", bufs=1))
        xkp = ctx.enter_context(tc.tile_pool(name="xkp", bufs=4))
        xqp = ctx.enter_context(tc.tile_pool(name="xqp", bufs=4))
        xvp = ctx.enter_context(tc.tile_pool(name="xvp", bufs=4))
        epool = ctx.enter_context(tc.tile_pool(name="epool", bufs=8))
        opool = ctx.enter_context(tc.tile_pool(name="opool", bufs=4))
        ospool = ctx.enter_context(tc.tile_pool(name="ospool", bufs=2))
        rpool = ctx.enter_context(tc.tile_pool(name="rpool", bufs=2))
        pp = ctx.enter_context(tc.tile_pool(name="pp", bufs=2, space="PSUM"))
        stp = ctx.enter_context(tc.tile_pool(name="stp", bufs=2, space="PSUM"))
        accp = ctx.enter_context(tc.tile_pool(name="accp", bufs=2, space="PSUM"))

        # ---------------- constants ----------------
        wq_sb = const.tile([128, 8, DL], BF16)
        wk_sb = const.tile([128, 8, DL], BF16)
        wv_sb = const.tile([128, 8, DL], BF16)
        wo_sb = const.tile([128, 2, D], BF16)
        mask_sb = const.tile([128, SEQ // 128], F32)
        ones_sb = const.tile([1, 64], F32)
        nc.vector.memset(ones_sb[:], 1.0)

        kTs = (
            proj.tile([128, SEQ], BF16, name="kT0"),
            proj.tile([128, SEQ], BF16, name="kT1"),
        )
        qTs = [
            [proj.tile([128, 512], BF16, name=f"qT{dm}_{qp}") for qp in range(4)]
            for dm in range(2)
        ]
        vaugs = [
            proj.tile([128, HL, 128], BF16, name=f"vaug{j}") for j in range(16)
        ]

        def vmemset(j):
            # col 0 = ones (denominator row); cols 1:64 zero; 64:128 = v
            # (written by vproj).  acc rows 1:63 are never read.
            nc.vector.memset(vaugs[j][:, :, 0:64], 0.0)
            nc.vector.memset(vaugs[j][:, :, 0:1], 1.0)

        # ---------------- DMA posts ----------------
        x_tiles = {}

        def dma_x(which, c, eng=None, split=False):
            src = {"k": xk, "q": xq, "v": xv}[which]
            pool = {"k": xkp, "q": xqp, "v": xvp}[which]
            eng = eng or nc.sync
            t = pool.tile([128, 8, 512], BF16, tag=f"x{which}", name=f"x{which}{c}")
            if split:
                eng.dma_start(t[:, 0:4, :], src[:, c * 8 : c * 8 + 4, :])
                eng.dma_start(t[:, 4:8, :], src[:, c * 8 + 4 : c * 8 + 8, :])
            else:
                eng.dma_start(t[:], src[:, c * 8 : (c + 1) * 8, :])
            x_tiles[(which, c)] = t

        # ---------------- background units (generators) ----------------
        def u_kproj(dm, c):
            x_t = x_tiles[("k", c)]
            ps = pp.tile([128, 512], F32, tag="pp", name=f"kproj{dm}{c}")
            for ki in range(8):
                nc.tensor.matmul(
                    ps[:],
                    lhsT=wk_sb[:, ki, dm * 128 : (dm + 1) * 128],
                    rhs=x_t[:, ki, :],
                    start=(ki == 0),
                    stop=(ki == 7),
                )
                if ki % 2 == 1 and ki < 7:
                    yield
            nc.vector.tensor_copy(kTs[dm][:, c * 512 : (c + 1) * 512], ps[:])

        def u_qproj(dm, qp):
            x_t = x_tiles[("q", qp)]
            ps = pp.tile([128, 512], F32, tag="pp", name=f"qproj{dm}{qp}")
            for ki in range(8):
                nc.tensor.matmul(
                    ps[:],
                    lhsT=wq_sb[:, ki, dm * 128 : (dm + 1) * 128],
                    rhs=x_t[:, ki, :],
                    start=(ki == 0),
                    stop=(ki == 7),
                )
                if ki % 2 == 1 and ki < 7:
                    yield
            nc.vector.tensor_copy(qTs[dm][qp][:], ps[:])

        def u_vproj(j):
            c, km = j // 4, j % 4
            x_t = x_tiles[("v", c)]
            ps = pp.tile([128, 512], F32, tag="pp", name=f"vproj{j}")
            for ki in range(8):
                nc.tensor.matmul(
                    ps[:, 0:DL],
                    lhsT=x_t[:, ki, km * 128 : (km + 1) * 128],
                    rhs=wv_sb[:, ki, :],
                    start=(ki == 0),
                    stop=(ki == 7),
                )
                if ki % 3 == 2 and ki < 7:
                    yield
            nc.vector.tensor_copy(
                vaugs[j][:, :, 64 : 64 + DH],
                ps[:, 0:DL].rearrange("p (h d) -> p h d", h=HL),
            )

        outTs = [opool.tile([128, 2, 512], BF16, name=f"outT{qp}") for qp in range(4)]

        def u_wo(qp, mq):
            qg = qp * 4 + mq
            o_sb = ospool.tile([128, D], BF16, tag="o", name=f"wo_o{qp}_{mq}")
            for oc in range(2):
                ps = pp.tile([128, 512], F32, tag="pp", name=f"wops{qp}_{mq}_{oc}")
                for kc in range(2):
                    nc.tensor.matmul(
                        ps[:],
                        lhsT=outTs[qp][:, kc, mq * 128 : (mq + 1) * 128],
                        rhs=wo_sb[:, kc, oc * 512 : (oc + 1) * 512],
                        start=(kc == 0),
                        stop=(kc == 1),
                    )
                nc.vector.tensor_scalar_mul(
                    o_sb[:, oc * 512 : (oc + 1) * 512], ps[:], mask_sb[:, qg : qg + 1]
                )
                if oc == 0:
                    yield
            nc.sync.dma_start(out_part[qg * 128 : (qg + 1) * 128, :], o_sb[:])

        def normalize(qp, hp, hi, acc_ps):
            acc_sb = rpool.tile([128, 512], F32, tag="accsb")
            nc.vector.tensor_copy(acc_sb[:], acc_ps[:])
            r_sb = rpool.tile([1, 512], F32, tag="r")
            nc.vector.reciprocal_approx_fast(out=r_sb[:], in_=acc_sb[0:1, :])
            rb_ps = pp.tile([64, 512], F32, tag="pp", name=f"rb{qp}_{hp}_{hi}")
            nc.tensor.matmul(
                rb_ps[:], lhsT=ones_sb[:], rhs=r_sb[:], start=True, stop=True
            )
            nc.vector.tensor_mul(
                outTs[qp][hi * 64 : (hi + 1) * 64, hp, :],
                acc_sb[64 : 64 + DH, :],
                rb_ps[:],
            )

        # ---------------- background queue machinery ----------------
        # Each entry: [deadline, not_before, generator].  pump() advances the
        # head generator by one yield (~2 matmuls) but not before its
        # not_before step (so eager pumping can't emit matmuls that would
        # block the PE FIFO waiting on a DMA that hasn't landed).
        # drain_due() completes all units whose deadline has arrived (called
        # after scores+exp, before PV, so a unit due at step s is fully
        # emitted before step s+1's scores).
        bg_q = []

        def enq(deadline, not_before, gen):
            bg_q.append([deadline, not_before, gen])

        def pump(n, step):
            while n > 0 and bg_q and bg_q[0][1] <= step:
                try:
                    next(bg_q[0][2])
                    n -= 1
                except StopIteration:
                    bg_q.pop(0)

        def drain_due(step):
            while bg_q and bg_q[0][0] <= step:
                for _ in bg_q[0][2]:
                    pass
                bg_q.pop(0)

        # fixed-step actions (DMA posts, vaug memsets) keyed by global step
        actions = {}

        def at_step(s, fn):
            actions.setdefault(s, []).append(fn)

        # ---------------- prologue ----------------
        vmemset(0)
        vmemset(1)
        vmemset(2)
        vmemset(3)
        # scalar ring: the critical first K/Q chunks (ACT idles until ~13us)
        dma_x("k", 0, eng=nc.scalar, split=True)
        dma_x("q", 0, eng=nc.scalar, split=True)
        # sync ring in need-order
        nc.sync.dma_start(wk_sb[:], wk[:])
        nc.sync.dma_start(wq_sb[:], wq[:])
        nc.sync.dma_start(wv_sb[:], wv[:])
        dma_x("v", 0)
        dma_x("k", 1)
        for g in u_kproj(0, 0):
            pass
        for g in u_qproj(0, 0):
            pass

        # staggered DMA posts (sync ring) and vaug memsets
        at_step(0, lambda: dma_x("v", 1))
        at_step(2, lambda: dma_x("k", 2))
        at_step(4, lambda: dma_x("v", 2))
        at_step(6, lambda: dma_x("k", 3))
        at_step(8, lambda: dma_x("v", 3))
        at_step(10, lambda: dma_x("q", 1))
        at_step(12, lambda: nc.sync.dma_start(wo_sb[:], wo[:]))
        at_step(12, lambda: nc.sync.dma_start(mask_sb[:], maskf[:]))
        at_step(13, lambda: dma_x("q", 2))
        at_step(15, lambda: dma_x("q", 3))
        for j in range(4, 16):
            at_step(j - 2, (lambda jj: lambda: vmemset(jj))(j))

        # background unit queue (insertion order = deadline order).
        # Block 0 PV emission is delayed by PV_DELAY steps, so vproj(j) only
        # needs to be emitted by step j+PV_DELAY-1 (capped at 15).
        PV_DELAY = 6
        enq(2, 1, u_kproj(0, 1))
        enq(5, 1, u_vproj(0))
        enq(5, 1, u_vproj(1))
        enq(5, 1, u_vproj(2))
        enq(6, 2, u_vproj(3))
        enq(6, 4, u_kproj(0, 2))
        enq(9, 4, u_vproj(4))
        enq(10, 4, u_vproj(5))
        enq(10, 7, u_kproj(0, 3))
        enq(11, 5, u_vproj(6))
        enq(12, 5, u_vproj(7))
        enq(13, 8, u_vproj(8))
        enq(14, 8, u_vproj(9))
        enq(15, 9, u_vproj(10))
        enq(15, 9, u_vproj(11))
        enq(15, 11, u_vproj(12))
        enq(15, 11, u_vproj(13))
        enq(15, 12, u_vproj(14))
        enq(15, 12, u_vproj(15))
        enq(15, 13, u_qproj(0, 1))
        enq(28, 17, u_kproj(1, 0))
        enq(31, 18, u_qproj(0, 2))
        enq(36, 19, u_kproj(1, 1))
        enq(44, 20, u_kproj(1, 2))
        enq(47, 21, u_qproj(0, 3))
        enq(52, 22, u_kproj(1, 3))
        enq(62, 24, u_qproj(1, 0))
        enq(78, 26, u_qproj(1, 1))
        enq(94, 28, u_qproj(1, 2))
        enq(110, 30, u_qproj(1, 3))

        # ---------------- main loop ----------------
        for block in range(8):
            hp, qp = block // 4, block % 4
            acc = [
                accp.tile([128, 512], F32, tag="acc", name=f"acc{hp}_{qp}_{i}")
                for i in range(2)
            ]
            pv_backlog = []

            def emit_pv(jj, e_tt):
                for hi in range(2):
                    nc.tensor.matmul(
                        acc[hi][:],
                        lhsT=vaugs[jj][:, 2 * hp + hi, :],
                        rhs=e_tt[:, hi * 512 : (hi + 1) * 512],
                        start=(jj == 0),
                        stop=(jj == 15),
                    )

            delay = PV_DELAY if block == 0 else 0
            for j in range(16):
                gs = block * 16 + j
                st = stp.tile([128, 1024], F32, tag="st")
                for hi in range(2):
                    r0 = hi * 64
                    nc.tensor.matmul(
                        st[:, hi * 512 : (hi + 1) * 512],
                        lhsT=kTs[hp][r0 : r0 + 64, j * 128 : (j + 1) * 128],
                        rhs=qTs[hp][qp][r0 : r0 + 64, :],
                        start=True,
                        stop=True,
                    )
                e_t = epool.tile([128, 1024], BF16, tag="e")
                nc.scalar.activation(out=e_t[:], in_=st[:], func=AF.Exp, scale=0.125)
                for fn in actions.get(gs, ()):
                    fn()
                drain_due(gs)
                pump(2 if gs < 64 else 1, gs)
                pv_backlog.append((j, e_t))
                if len(pv_backlog) > delay:
                    emit_pv(*pv_backlog.pop(0))
            for jj, e_tt in pv_backlog:
                emit_pv(jj, e_tt)
            for hi in range(2):
                normalize(qp, hp, hi, acc[hi])
            if hp == 1 and qp < 3:
                for mq in range(4):
                    enq(127, 0, u_wo(qp, mq))
        drain_due(1000)
        # tail: W_o of the last qp
        for mq in range(4):
            for g in u_wo(3, mq):
                pass

    nc.compile()
    return nc


def _get_program():
    global _PROGRAM
    if _PROGRAM is None:
        _PROGRAM = build_program()
    return _PROGRAM


def _pack_x(x):
    # [2048 seq, 1024 model] -> [p, c*8+a, s]: element (model a*128+p, seq c*512+s)
    xt = np.ascontiguousarray(x.T).reshape(8, 128, 4, 512)
    return np.ascontiguousarray(xt.transpose(1, 2, 0, 3).reshape(128, 32, 512))


def _pack_w(wt, a):
    # [a*128 contraction, d] -> [p, a, d]: element (a*128+p, d)
    return np.ascontiguousarray(wt.reshape(a, 128, wt.shape[1]).transpose(1, 0, 2))


def make_in_maps(Q, K, V, mask, W_q, W_k, W_v, W_o):
    bf = ml_dtypes.bfloat16
    Q, K, V = (np.asarray(a, np.float32) for a in (Q, K, V))
    W_q, W_k, W_v, W_o = (np.asarray(a, np.float32) for a in (W_q, W_k, W_v, W_o))
    mask = np.asarray(mask)
    in_maps = []
    for core in range(NCORES):
        b, hg = core // 4, core % 4
        c0 = hg * DL
        in_maps.append(
            {
                "xq": _pack_x(Q[b]).astype(bf),
                "xk": _pack_x(K[b]).astype(bf),
                "xv": _pack_x(V[b]).astype(bf),
                "wq": _pack_w(W_q[c0 : c0 + DL, :].T, 8).astype(bf),
                "wk": _pack_w(W_k[c0 : c0 + DL, :].T, 8).astype(bf),
                "wv": _pack_w(W_v[c0 : c0 + DL, :].T, 8).astype(bf),
                "wo": _pack_w(W_o[:, c0 : c0 + DL].T, 2).astype(bf),
                "maskf": np.ascontiguousarray(
                    mask[b].reshape(SEQ // 128, 128).T
                ).astype(np.float32),
            }
        )
    return in_maps


def gather(results):
    out = np.zeros((B, SEQ, D), np.float32)
    for core in range(NCORES):
        out[core // 4] += results[core]["out_part"].astype(np.float32)
    return out


def kernel(Q, K, V, mask, W_q, W_k, W_v, W_o):
    from concourse.bass_utils import run_bass_kernel_spmd

    nc = _get_program()
    in_maps = make_in_maps(Q, K, V, mask, W_q, W_k, W_v, W_o)
    res = run_bass_kernel_spmd(nc, in_maps, list(range(NCORES))).results
    return gather(res)


# revision 27
# speedup vs baseline: 1.0903x; 1.0903x over previous
"""MultiHeadCrossAttention kernel for 8 trn2 NeuronCores.

Reference computation (fp32, per batch b):
    q = Q[b] @ W_q.T ; k = K[b] @ W_k.T ; v = V[b] @ W_v.T      (heads on columns)
    per head h: S = (q_h @ k_h.T) / 8 ; E = exp(S); A = E / E.sum(-1)
    out[b] = concat_h(A @ v_h) @ W_o.T ; rows with mask==0 zeroed

Sharding: 8 cores = (batch b in {0,1}) x (head-group hg in {0..3}, 4 heads each).
Each core computes a partial output  out_part[b] = concat(heads hg) @ W_o[:, cols].T
and the host sums the 4 partials per batch (bf16 partials, fp32 host sum).

Design: the kernel is ScalarE-bound (exp over 4 heads x 2048 x 2048 = 16.8M
elements at 1 elem/lane/cycle @ 1.2 GHz ~= 147 us).  Everything else is
scheduled to hide under the exp stream:
  - The attention j-loop emits scores (row-tiled concurrent matmul pair, two
    heads at PE rows 0:64/64:128), the exp ACTIVATE, then background work,
    then the PV matmuls (augmented-V: stationary col 0 = ones accumulates the
    softmax denominator in PSUM row 0).
  - Background work (remaining projections, W_o, DMA posts) is emitted as
    generator "units" that yield every ~2 matmuls, pumped at most 2 yields per
    j-step so the PE FIFO never delays the next scores pair by more than
    ~0.9us.  Deadline fields force-drain units whose results the next step
    needs.
  - Deep e-tile buffering lets ScalarE run ahead of PV when the PE transiently
    falls behind (e.g. late V-chunk DMAs).
  - DMA posts cost ~0.65us each on the posting engine's queue and transfers
    are slow (~130-250 GB/s effective), so the first K/Q chunks go on the
    scalar ring (idle before the exp stream starts) while everything else
    staggers through the sync ring in need-order.
  - Reciprocal broadcast across partitions via a K=1 PE outer-product.
"""

import numpy as np
import ml_dtypes

import concourse.bass as bass
import concourse.bacc as bacc
import concourse.mybir as mybir
import concourse.tile as tile
from contextlib import ExitStack

F32 = mybir.dt.float32
BF16 = mybir.dt.bfloat16
AF = mybir.ActivationFunctionType

B = 2
SEQ = 2048          # Sq == Sk
D = 1024            # model dim
DL = 256            # local head dims per core (4 heads x 64)
HL = 4              # local heads
DH = 64             # head dim
NCORES = 8

_PROGRAM = None


def build_program():
    nc = bacc.Bacc("TRN2", target_bir_lowering=False)

    # x inputs host-packed as [p, c*8+a, s]: element (model a*128+p, seq c*512+s)
    xq = nc.declare_dram_parameter("xq", [128, 32, 512], BF16, isOutput=False)
    xk = nc.declare_dram_parameter("xk", [128, 32, 512], BF16, isOutput=False)
    xv = nc.declare_dram_parameter("xv", [128, 32, 512], BF16, isOutput=False)
    # weights host-packed [p, a, d]: element (model a*128+p, d)
    wq = nc.declare_dram_parameter("wq", [128, 8, DL], BF16, isOutput=False)
    wk = nc.declare_dram_parameter("wk", [128, 8, DL], BF16, isOutput=False)
    wv = nc.declare_dram_parameter("wv", [128, 8, DL], BF16, isOutput=False)
    wo = nc.declare_dram_parameter("wo", [128, 2, D], BF16, isOutput=False)
    maskf = nc.declare_dram_parameter("maskf", [1, 2048], BF16, isOutput=False)
    out_part = nc.declare_dram_parameter("out_part", [SEQ, D], BF16, isOutput=True)

    with tile.TileContext(nc) as tc, ExitStack() as ctx:
        const = ctx.enter_context(tc.tile_pool(name="const", bufs=1))
        proj = ctx.enter_context(tc.tile_pool(name="proj", bufs=1))
        xkp = ctx.enter_context(tc.tile_pool(name="xkp", bufs=4))
        xqp = ctx.enter_context(tc.tile_pool(name="xqp", bufs=4))
        xvp = ctx.enter_context(tc.tile_pool(name="xvp", bufs=3))
        epool = ctx.enter_context(tc.tile_pool(name="epool", bufs=10))
        opool = ctx.enter_context(tc.tile_pool(name="opool", bufs=4))
        ospool = ctx.enter_context(tc.tile_pool(name="ospool", bufs=2))
        rpool = ctx.enter_context(tc.tile_pool(name="rpool", bufs=2))
        pp = ctx.enter_context(tc.tile_pool(name="pp", bufs=2, space="PSUM"))
        stp = ctx.enter_context(tc.tile_pool(name="stp", bufs=2, space="PSUM"))
        accp = ctx.enter_context(tc.tile_pool(name="accp", bufs=2, space="PSUM"))

        # ---------------- constants ----------------
        wq_sb = const.tile([128, 8, DL], BF16)
        wk_sb = const.tile([128, 8, DL], BF16)
        wv_sb = const.tile([128, 8, DL], BF16)
        wo_sb = const.tile([128, 2, D], BF16)
        mask_sb = const.tile([1, 2048], BF16)
        dummy_sb = const.tile([1, 512], BF16)
        ones_sb = const.tile([1, 64], BF16)
        nc.vector.memset(ones_sb[:], 1.0)
        nc.vector.memset(dummy_sb[:], 1.0)

        kTs = (
            proj.tile([128, SEQ], BF16, name="kT0"),
            proj.tile([128, SEQ], BF16, name="kT1"),
        )
        qTs = [
            [proj.tile([128, 512], BF16, name=f"qT{dm}_{qp}") for qp in range(4)]
            for dm in range(2)
        ]
        vaugs = [
            proj.tile([128, HL, 128], BF16, name=f"vaug{j}") for j in range(16)
        ]

        def vmemset(j):
            # col 0 = ones (denominator row); cols 1:64 zero; 64:128 = v
            # (written by vproj).  acc rows 1:63 are never read.
            nc.vector.memset(vaugs[j][:, :, 0:64], 0.0)
            nc.vector.memset(vaugs[j][:, :, 0:1], 1.0)

        # ---------------- DMA posts ----------------
        x_tiles = {}

        def dma_x(which, c, eng=None, split=False):
            src = {"k": xk, "q": xq, "v": xv}[which]
            pool = {"k": xkp, "q": xqp, "v": xvp}[which]
            eng = eng or nc.sync
            t = pool.tile([128, 8, 512], BF16, tag=f"x{which}", name=f"x{which}{c}")
            if split:
                eng.dma_start(t[:, 0:4, :], src[:, c * 8 : c * 8 + 4, :])
                eng.dma_start(t[:, 4:8, :], src[:, c * 8 + 4 : c * 8 + 8, :])
            else:
                eng.dma_start(t[:], src[:, c * 8 : (c + 1) * 8, :])
            x_tiles[(which, c)] = t

        # ---------------- background units (generators) ----------------
        def u_kproj(dm, c):
            x_t = x_tiles[("k", c)]
            ps = pp.tile([128, 512], F32, tag="pp", name=f"kproj{dm}{c}")
            for ki in range(8):
                nc.tensor.matmul(
                    ps[:],
                    lhsT=wk_sb[:, ki, dm * 128 : (dm + 1) * 128],
                    rhs=x_t[:, ki, :],
                    start=(ki == 0),
                    stop=(ki == 7),
                )
                if ki % 2 == 1 and ki < 7:
                    yield
            nc.vector.tensor_copy(kTs[dm][:, c * 512 : (c + 1) * 512], ps[:])

        def u_qproj(dm, qp):
            x_t = x_tiles[("q", qp)]
            ps = pp.tile([128, 512], F32, tag="pp", name=f"qproj{dm}{qp}")
            for ki in range(8):
                nc.tensor.matmul(
                    ps[:],
                    lhsT=wq_sb[:, ki, dm * 128 : (dm + 1) * 128],
                    rhs=x_t[:, ki, :],
                    start=(ki == 0),
                    stop=(ki == 7),
                )
                if ki % 2 == 1 and ki < 7:
                    yield
            nc.vector.tensor_copy(qTs[dm][qp][:], ps[:])

        def u_vproj(j):
            c, km = j // 4, j % 4
            x_t = x_tiles[("v", c)]
            ps = pp.tile([128, 512], F32, tag="pp", name=f"vproj{j}")
            for ki in range(8):
                nc.tensor.matmul(
                    ps[:, 0:DL],
                    lhsT=x_t[:, ki, km * 128 : (km + 1) * 128],
                    rhs=wv_sb[:, ki, :],
                    start=(ki == 0),
                    stop=(ki == 7),
                )
                if ki % 3 == 2 and ki < 7:
                    yield
            nc.vector.tensor_copy(
                vaugs[j][:, :, 64 : 64 + DH],
                ps[:, 0:DL].rearrange("p (h d) -> p h d", h=HL),
            )

        outTs = [opool.tile([128, 2, 512], BF16, name=f"outT{qp}") for qp in range(4)]

        def u_wo(qp, mq):
            qg = qp * 4 + mq
            o_sb = ospool.tile([128, D], BF16, tag="o", name=f"wo_o{qp}_{mq}")
            for oc in range(2):
                ps = pp.tile([128, 512], F32, tag="pp", name=f"wops{qp}_{mq}_{oc}")
                for kc in range(2):
                    nc.tensor.matmul(
                        ps[:],
                        lhsT=outTs[qp][:, kc, mq * 128 : (mq + 1) * 128],
                        rhs=wo_sb[:, kc, oc * 512 : (oc + 1) * 512],
                        start=(kc == 0),
                        stop=(kc == 1),
                    )
                nc.vector.tensor_copy(o_sb[:, oc * 512 : (oc + 1) * 512], ps[:])
                if oc == 0:
                    yield
            nc.sync.dma_start(out_part[qg * 128 : (qg + 1) * 128, :], o_sb[:])

        def normalize(qp, hp, hi, acc_ps):
            r_sb = rpool.tile([1, 512], F32, tag="r")
            nc.vector.reciprocal_approx_fast(out=r_sb[:], in_=acc_ps[0:1, :])
            acc_sb = rpool.tile([128, 512], F32, tag="accsb")
            nc.vector.tensor_copy(acc_sb[:], acc_ps[:])
            r_bf = rpool.tile([1, 512], BF16, tag="rbf")
            nc.vector.tensor_mul(
                r_bf[:], r_sb[:], mask_sb[0:1, qp * 512 : (qp + 1) * 512]
            )
            rb_ps = pp.tile([64, 512], F32, tag="pp", name=f"rb{qp}_{hp}_{hi}")
            nc.tensor.matmul(
                rb_ps[:], lhsT=ones_sb[:], rhs=r_bf[:], start=True, stop=True
            )
            nc.vector.tensor_mul(
                outTs[qp][hi * 64 : (hi + 1) * 64, hp, :],
                acc_sb[64 : 64 + DH, :],
                rb_ps[:],
            )

        # ---------------- background queue machinery ----------------
        # Each entry: [deadline, not_before, generator].  pump() advances the
        # head generator by one yield (~2 matmuls) but not before its
        # not_before step (so eager pumping can't emit matmuls that would
        # block the PE FIFO waiting on a DMA that hasn't landed).
        # drain_due() completes all units whose deadline has arrived.
        bg_q = []

        def enq(deadline, not_before, gen):
            bg_q.append([deadline, not_before, gen])

        def pump(n, step):
            while n > 0 and bg_q and bg_q[0][1] <= step:
                try:
                    next(bg_q[0][2])
                    n -= 1
                except StopIteration:
                    bg_q.pop(0)

        def drain_due(step):
            while bg_q and bg_q[0][0] <= step:
                for _ in bg_q[0][2]:
                    pass
                bg_q.pop(0)

        # fixed-step actions (DMA posts, vaug memsets) keyed by global step
        actions = {}

        def at_step(s, fn):
            actions.setdefault(s, []).append(fn)

        def u_kproj0_sub(s):
            # first K chunk in 128-kpos sub-chunks so scores(0,0) only waits
            # on 0.25 MB of K data
            x_t = x_tiles[("k", 0)]
            ps = pp.tile([128, 512], F32, tag="pp", name=f"kp0s{s}")
            for ki in range(8):
                nc.tensor.matmul(
                    ps[:, 0:128],
                    lhsT=wk_sb[:, ki, 0:128],
                    rhs=x_t[:, ki, s * 128 : (s + 1) * 128],
                    start=(ki == 0),
                    stop=(ki == 7),
                )
                if ki == 3 and s > 0:
                    yield
            nc.vector.tensor_copy(
                kTs[0][:, s * 128 : (s + 1) * 128], ps[:, 0:128]
            )

        # ---------------- prologue ----------------
        for j in range(4):
            vmemset(j)
        # scalar ring: first K sub-chunk, then the first Q chunk
        t = xkp.tile([128, 8, 512], BF16, tag="xk", name="xk0")
        nc.scalar.dma_start(t[:, :, 0:128], xk[:, 0:8, 0:128])
        x_tiles[("k", 0)] = t
        dma_x("q", 0, eng=nc.scalar, split=True)
        nc.scalar.dma_start(t[:, :, 128:512], xk[:, 0:8, 128:512])
        # sync ring: just the first two weights; everything else staggers in
        nc.sync.dma_start(wk_sb[:], wk[:])
        nc.sync.dma_start(wq_sb[:], wq[:])
        # PE warmup: dummy K=1 matmuls get the HAM clock to 2.4 GHz while the
        # first DMAs land, so the first projections run warm
        for w in range(16):
            wm_ps = pp.tile([64, 512], F32, tag="pp", name=f"warm{w}")
            nc.tensor.matmul(
                wm_ps[:], lhsT=ones_sb[:], rhs=dummy_sb[:], start=True, stop=True
            )
        for g in u_kproj0_sub(0):
            pass
        for g in u_qproj(0, 0):
            pass

        # staggered DMA posts (sync ring) in need-order
        at_step(0, lambda: dma_x("k", 1))
        at_step(0, lambda: nc.sync.dma_start(wv_sb[:], wv[:]))
        at_step(0, lambda: dma_x("v", 0))
        at_step(1, lambda: dma_x("k", 2))
        at_step(3, lambda: dma_x("v", 1))
        at_step(4, lambda: dma_x("k", 3))
        at_step(5, lambda: dma_x("q", 1))
        at_step(6, lambda: dma_x("v", 2))
        at_step(8, lambda: dma_x("v", 3))
        at_step(10, lambda: dma_x("q", 2))
        at_step(12, lambda: nc.sync.dma_start(wo_sb[:], wo[:]))
        at_step(12, lambda: nc.sync.dma_start(mask_sb[:], maskf[:]))
        at_step(14, lambda: dma_x("q", 3))
        for j in range(4, 16):
            at_step(j, (lambda jj: lambda: vmemset(jj))(j))

        # background unit queue, sorted by deadline
        enq(0, 0, u_kproj0_sub(1))
        enq(1, 0, u_kproj0_sub(2))
        enq(2, 0, u_kproj0_sub(3))
        enq(3, 2, u_kproj(0, 1))
        enq(5, 4, u_vproj(0))
        enq(6, 4, u_vproj(1))
        enq(7, 4, u_vproj(2))
        enq(7, 5, u_kproj(0, 2))
        enq(8, 4, u_vproj(3))
        enq(9, 8, u_vproj(4))
        enq(10, 8, u_vproj(5))
        enq(11, 8, u_vproj(6))
        enq(11, 9, u_kproj(0, 3))
        enq(12, 8, u_vproj(7))
        enq(13, 11, u_vproj(8))
        enq(14, 11, u_vproj(9))
        enq(15, 11, u_vproj(10))
        enq(15, 11, u_vproj(11))
        enq(15, 12, u_qproj(0, 1))
        enq(15, 13, u_vproj(12))
        enq(15, 13, u_vproj(13))
        enq(16, 13, u_vproj(14))
        enq(16, 13, u_vproj(15))
        enq(31, 20, u_qproj(0, 2))
        enq(47, 26, u_qproj(0, 3))
        enq(50, 34, u_kproj(1, 0))
        enq(53, 38, u_kproj(1, 1))
        enq(56, 42, u_kproj(1, 2))
        enq(59, 46, u_kproj(1, 3))
        enq(62, 50, u_qproj(1, 0))
        enq(79, 56, u_qproj(1, 1))
        enq(95, 60, u_qproj(1, 2))
        enq(111, 64, u_qproj(1, 3))

        # ---------------- main loop ----------------
        # PV emission runs PV_DELAY steps behind the exp stream; the tail of
        # each block's PV plus its normalization carries into the next block's
        # first steps (2 items/step) so scores/exp never sit behind them.
        PV_DELAY0, PV_DELAY = 6, 4
        acc_box = {}
        carry = []

        def make_pv(block, jj, e_tt):
            hp = block // 4

            def f():
                if block not in acc_box:
                    acc_box[block] = [
                        accp.tile([128, 512], F32, tag="acc", name=f"acc{block}_{i}")
                        for i in range(2)
                    ]
                acc = acc_box[block]
                for hi in range(2):
                    nc.tensor.matmul(
                        acc[hi][:],
                        lhsT=vaugs[jj][:, 2 * hp + hi, :],
                        rhs=e_tt[:, hi * 512 : (hi + 1) * 512],
                        start=(jj == 0),
                        stop=(jj == 15),
                    )

            return f

        def make_norm(block, hi):
            hp, qp = block // 4, block % 4

            def f():
                normalize(qp, hp, hi, acc_box[block][hi])
                if hi == 1 and hp == 1 and qp < 3:
                    for mq in range(4):
                        enq(127, 0, u_wo(qp, mq))

            return f

        pv_backlog = []
        for block in range(8):
            hp, qp = block // 4, block % 4
            delay = PV_DELAY0 if block == 0 else PV_DELAY
            for j in range(16):
                gs = block * 16 + j
                st = stp.tile([128, 1024], F32, tag="st")
                for hi in range(2):
                    r0 = hi * 64
                    nc.tensor.matmul(
                        st[:, hi * 512 : (hi + 1) * 512],
                        lhsT=kTs[hp][r0 : r0 + 64, j * 128 : (j + 1) * 128],
                        rhs=qTs[hp][qp][r0 : r0 + 64, :],
                        start=True,
                        stop=True,
                    )
                e_t = epool.tile([128, 1024], BF16, tag="e")
                nc.scalar.activation(out=e_t[:], in_=st[:], func=AF.Exp, scale=0.125)
                for fn in actions.get(gs, ()):
                    fn()
                nc_carry = 0
                while carry and nc_carry < 2:
                    carry.pop(0)()
                    nc_carry += 1
                drain_due(gs)
                pump(max(0, 2 - nc_carry) if gs < 64 else max(0, 1 - nc_carry), gs)
                pv_backlog.append(make_pv(block, j, e_t))
                if len(pv_backlog) > delay:
                    pv_backlog.pop(0)()
            # defer this block's PV tail + normalization into the next block
            carry.extend(pv_backlog)
            pv_backlog = []
            carry.append(make_norm(block, 0))
            carry.append(make_norm(block, 1))
        # tail
        for f in carry:
            f()
        drain_due(1000)
        for mq in range(4):
            for g in u_wo(3, mq):
                pass

    nc.compile()
    return nc


def _get_program():
    global _PROGRAM
    if _PROGRAM is None:
        _PROGRAM = build_program()
    return _PROGRAM


def _pack_x(x):
    # [2048 seq, 1024 model] -> [p, c*8+a, s]: element (model a*128+p, seq c*512+s)
    xt = np.ascontiguousarray(x.T).reshape(8, 128, 4, 512)
    return np.ascontiguousarray(xt.transpose(1, 2, 0, 3).reshape(128, 32, 512))


def _pack_w(wt, a):
    # [a*128 contraction, d] -> [p, a, d]: element (a*128+p, d)
    return np.ascontiguousarray(wt.reshape(a, 128, wt.shape[1]).transpose(1, 0, 2))


def make_in_maps(Q, K, V, mask, W_q, W_k, W_v, W_o):
    bf = ml_dtypes.bfloat16
    Q, K, V = (np.asarray(a, np.float32) for a in (Q, K, V))
    W_q, W_k, W_v, W_o = (np.asarray(a, np.float32) for a in (W_q, W_k, W_v, W_o))
    mask = np.asarray(mask)
    in_maps = []
    for core in range(NCORES):
        b, hg = core // 4, core % 4
        c0 = hg * DL
        in_maps.append(
            {
                "xq": _pack_x(Q[b]).astype(bf),
                "xk": _pack_x(K[b]).astype(bf),
                "xv": _pack_x(V[b]).astype(bf),
                "wq": _pack_w(W_q[c0 : c0 + DL, :].T, 8).astype(bf),
                "wk": _pack_w(W_k[c0 : c0 + DL, :].T, 8).astype(bf),
                "wv": _pack_w(W_v[c0 : c0 + DL, :].T, 8).astype(bf),
                "wo": _pack_w(W_o[:, c0 : c0 + DL].T, 2).astype(bf),
                "maskf": np.ascontiguousarray(mask[b].reshape(1, 2048)).astype(bf),
            }
        )
    return in_maps


def gather(results):
    out = np.zeros((B, SEQ, D), np.float32)
    for core in range(NCORES):
        out[core // 4] += results[core]["out_part"].astype(np.float32)
    return out


def kernel(Q, K, V, mask, W_q, W_k, W_v, W_o):
    from concourse.bass_utils import run_bass_kernel_spmd

    nc = _get_program()
    in_maps = make_in_maps(Q, K, V, mask, W_q, W_k, W_v, W_o)
    res = run_bass_kernel_spmd(nc, in_maps, list(range(NCORES))).results
    return gather(res)
